# revision 1
# baseline (speedup 1.0000x reference)
"""OCCNet (Instant-NGP hash-grid encoder + tiny MLP) on 8 TRN2 NeuronCores.

Data-parallel over points: each core processes 131072 points with the 16
hash tables (bf16, replicated into every SBUF partition) and the MLP
weights replicated. Gathers run on GPSIMD ap_gather; index/weight math on
DVE; MLP on PE (transpose + 3 matmuls) with ACT relu/sigmoid.
"""
import os
import sys
import types

sys.path.insert(0, "/opt/trn_rl_repo")

import numpy as np

import concourse.bass as bass
import concourse.bacc as bacc
import concourse.mybir as mybir
import concourse.tile as tile
from concourse.masks import make_identity

# ---------------------------------------------------------------- problem dims
NUM_LODS = 16
FEAT_DIM = 4
TABLE_SIZE = 8192
N_PTS = 1048576
N_CORES = 8
N_CORE = N_PTS // N_CORES          # 131072 points per core

_min_res = 16
_b = np.exp((np.log(2.0 ** 19) - np.log(16.0)) / 15.0)
LODS = [int(1 + np.floor(_min_res * _b ** l)) for l in range(NUM_LODS)]

P1 = 2654435761
P2 = 805459861
P1L = P1 & 8191
P2L = P2 & 8191
P1H = (64 * P1L) & 8191
P2H = (64 * P2L) & 8191

TILE_PTS = 2048                    # points per gather tile
NI = TILE_PTS                      # ap_gather num_idxs per gpsimd core
GP = TILE_PTS // 8                 # points per 16-partition group (256)
KP = GP // 16                      # points per partition, wrap layout (16)
N_TILES = N_CORE // TILE_PTS       # 64
SUP = 2048                         # MLP super-tile
N_SUP = N_CORE // SUP              # 16

F32 = mybir.dt.float32
BF16 = mybir.dt.bfloat16
I32 = mybir.dt.int32
I16 = mybir.dt.int16
TT = mybir.AluOpType


def apz(a, dims, off=0):
    """Build an AP on the same tensor with explicit [step, count] dims."""
    return bass.AP(a.tensor, a.offset + off, [list(d) for d in dims])


def _axon_boot():
    import antenv
    if getattr(antenv, "axon_hooks", None) is None:
        mod = types.ModuleType("antenv.axon_hooks")
        mod._hook = None
        mod.set_axon_ntff_profile_hook = lambda h: setattr(mod, "_hook", h)
        mod.get_axon_ntff_profile_hook = lambda: mod._hook
        sys.modules["antenv.axon_hooks"] = mod
        antenv.axon_hooks = mod
        try:
            from trn_agent_boot.trn_boot import _ntff_profile_via_ctypes
            mod._hook = _ntff_profile_via_ctypes("/opt/axon/libaxon_pjrt.so")
        except Exception:
            pass
    import concourse.bass_utils as bass_utils
    bass_utils.upload_artifacts = lambda tmpdir: "local://" + tmpdir


def build(n_tiles=N_TILES, n_sup=N_SUP):
    nc = bacc.Bacc("TRN2", target_bir_lowering=False, debug=False,
                   num_devices=N_CORES)

    pts = nc.dram_tensor("pts", [N_CORE, 3], F32, kind="ExternalInput")
    ptsw = nc.dram_tensor("ptsw", [N_CORE, 3], F32, kind="ExternalInput")
    tabs = nc.dram_tensor("tabs", [NUM_LODS, 1, TABLE_SIZE * FEAT_DIM], BF16,
                          kind="ExternalInput")
    w0 = nc.dram_tensor("w0", [64, 64], F32, kind="ExternalInput")
    b0 = nc.dram_tensor("b0", [64, 1], F32, kind="ExternalInput")
    w1 = nc.dram_tensor("w1", [64, 64], F32, kind="ExternalInput")
    b1 = nc.dram_tensor("b1", [64, 1], F32, kind="ExternalInput")
    w2 = nc.dram_tensor("w2", [64, 1], F32, kind="ExternalInput")
    b2 = nc.dram_tensor("b2", [1, 1], F32, kind="ExternalInput")
    coefh = nc.dram_tensor("coefh", [1, 3], F32, kind="ExternalInput")
    coefl = nc.dram_tensor("coefl", [1, 3], F32, kind="ExternalInput")
    cadd = nc.dram_tensor("cadd", [1, 3], I32, kind="ExternalInput")
    doffs = nc.dram_tensor("doffs", [1, 8], F32, kind="ExternalInput")

    out = nc.dram_tensor("out", [N_CORE, 1], F32, kind="ExternalOutput")
    dbg_l = int(os.environ.get("DBG_L", "-1"))
    if dbg_l >= 0:
        d_pw = nc.dram_tensor("d_pw", [128, KP * 3], F32, kind="ExternalOutput")
        d_pr = nc.dram_tensor("d_pr", [128, GP * 3], F32, kind="ExternalOutput")
        d_idx = nc.dram_tensor("d_idx", [128, NI // 16], I16, kind="ExternalOutput")
        d_gt = nc.dram_tensor("d_gt", [128, NI * FEAT_DIM], BF16, kind="ExternalOutput")
        d_w8 = nc.dram_tensor("d_w8", [128, 8 * GP], F32, kind="ExternalOutput")
        d_v13 = nc.dram_tensor("d_v13", [128, KP * 3], I32, kind="ExternalOutput")
        d_vh = nc.dram_tensor("d_vh", [128, KP * 3], I32, kind="ExternalOutput")
        d_hb = nc.dram_tensor("d_hb", [128, KP * 3], I32, kind="ExternalOutput")
    fkind = "ExternalOutput" if os.environ.get("DBG_FEATS") else "Internal"
    feats_d = nc.dram_tensor("feats", [NUM_LODS, N_CORE, FEAT_DIM], F32,
                             kind=fkind)

    with tile.TileContext(nc) as tc:
        with tc.tile_pool(name="const", bufs=1) as cpool, \
             tc.tile_pool(name="tab", bufs=1) as tabpool, \
             tc.tile_pool(name="ptsp", bufs=2) as ptspool, \
             tc.tile_pool(name="wk", bufs=1) as wkpool, \
             tc.tile_pool(name="gth", bufs=2) as gpool, \
             tc.tile_pool(name="mlp", bufs=2) as mpool, \
             tc.tile_pool(name="ps", bufs=2, space="PSUM") as pspool:

            coefh_t = cpool.tile([128, 3], F32)
            nc.sync.dma_start(out=coefh_t[:], in_=coefh[:].to_broadcast((128, 3)))
            coefl_t = cpool.tile([128, 3], F32)
            nc.sync.dma_start(out=coefl_t[:], in_=coefl[:].to_broadcast((128, 3)))
            cadd_t = cpool.tile([128, 3], I32)
            nc.sync.dma_start(out=cadd_t[:], in_=cadd[:].to_broadcast((128, 3)))
            doffs_t = cpool.tile([128, 8], F32)
            nc.sync.dma_start(out=doffs_t[:], in_=doffs[:].to_broadcast((128, 8)))

            # ---------------- phase A: encode all LODs ----------------
            for l in range(NUM_LODS):
                res = LODS[l]
                dense = res ** 3 <= TABLE_SIZE
                tab_t = tabpool.tile([128, TABLE_SIZE * FEAT_DIM], BF16)
                nc.sync.dma_start(
                    out=tab_t[:],
                    in_=tabs[l].to_broadcast((128, TABLE_SIZE * FEAT_DIM)))

                for t in range(n_tiles):
                    p0 = t * TILE_PTS
                    # wrap-layout points: partition 16g+p <- group g pts
                    # {k*16+p}, laid out [k, xyz]
                    pw = ptspool.tile([128, KP * 3], F32, tag="pw")
                    nc.sync.dma_start(
                        out=pw[:],
                        in_=ptsw[p0:p0 + TILE_PTS, :].rearrange(
                            "(q k) c -> q (k c)", q=128))
                    # replicated points: every partition of group g holds
                    # all GP points of group g
                    pr = ptspool.tile([128, GP * 3], F32, tag="pr")
                    for g in range(8):
                        nc.sync.dma_start(
                            out=pr[g * 16:(g + 1) * 16, :],
                            in_=pts[p0 + g * GP:p0 + (g + 1) * GP, :]
                            .rearrange("(o k) c -> o (k c)", o=1)
                            .to_broadcast((16, GP * 3)))

                    # =========== wrap side: corner indices ===========
                    npw = KP * 3
                    posw = wkpool.tile([128, npw], F32, tag="posw")
                    nc.vector.tensor_scalar(
                        out=posw[:], in0=pw[:], scalar1=float(res - 1),
                        scalar2=None, op0=TT.mult)
                    rwi = wkpool.tile([128, npw], I32, tag="rwi")
                    nc.vector.tensor_copy(out=rwi[:], in_=posw[:])
                    rwf = wkpool.tile([128, npw], F32, tag="rwf")
                    nc.vector.tensor_copy(out=rwf[:], in_=rwi[:])
                    gw_ = wkpool.tile([128, npw], F32, tag="gw_")
                    nc.vector.tensor_tensor(
                        out=gw_[:], in0=rwf[:], in1=posw[:], op=TT.is_gt)
                    nc.vector.tensor_tensor(
                        out=rwf[:], in0=rwf[:], in1=gw_[:], op=TT.subtract)
                    nc.vector.tensor_scalar(
                        out=rwf[:], in0=rwf[:], scalar1=float(res - 2),
                        scalar2=None, op0=TT.min)
                    ciw = wkpool.tile([128, npw], I32, tag="ciw")
                    nc.vector.tensor_copy(out=ciw[:], in_=rwf[:])
                    idxw = wkpool.tile([128, NI // 16], I16, tag="idxw")
                    if dense:
                        cfw = wkpool.tile([128, npw], F32, tag="cfw")
                        nc.vector.tensor_copy(out=cfw[:], in_=ciw[:])
                        fl = wkpool.tile([128, KP], F32, tag="fl")
                        nc.vector.tensor_scalar(
                            out=fl[:], in0=cfw[:, 0::3], scalar1=float(res),
                            scalar2=None, op0=TT.mult)
                        nc.vector.tensor_tensor(
                            out=fl[:], in0=fl[:], in1=cfw[:, 1::3], op=TT.add)
                        nc.vector.tensor_scalar(
                            out=fl[:], in0=fl[:], scalar1=float(res),
                            scalar2=None, op0=TT.mult)
                        nc.vector.tensor_tensor(
                            out=fl[:], in0=fl[:], in1=cfw[:, 2::3], op=TT.add)
                        f8 = wkpool.tile([128, 8 * KP], F32, tag="f8")
                        flp = fl[:].ap[0][0]
                        dop = doffs_t[:].ap[0][0]
                        f8p = f8[:].ap[0][0]
                        nc.vector.tensor_tensor(
                            out=apz(f8[:], [[f8p, 128], [KP, 8], [1, KP]]),
                            in0=apz(fl[:], [[flp, 128], [0, 8], [1, KP]]),
                            in1=apz(doffs_t[:], [[dop, 128], [1, 8], [0, KP]]),
                            op=TT.add)
                        nc.vector.tensor_copy(out=idxw[:], in_=f8[:])
                    else:
                        v13 = wkpool.tile([128, npw], I32, tag="v13")
                        nc.vector.tensor_scalar(
                            out=v13[:], in0=ciw[:], scalar1=8191, scalar2=None,
                            op0=TT.bitwise_and)
                        vh = wkpool.tile([128, npw], I32, tag="vh")
                        nc.vector.tensor_scalar(
                            out=vh[:], in0=v13[:], scalar1=8128, scalar2=None,
                            op0=TT.bitwise_and)
                        vl = wkpool.tile([128, npw], I32, tag="vl")
                        nc.vector.tensor_scalar(
                            out=vl[:], in0=v13[:], scalar1=63, scalar2=None,
                            op0=TT.bitwise_and)
                        vhf = wkpool.tile([128, npw], F32, tag="vhf")
                        nc.vector.tensor_copy(out=vhf[:], in_=vh[:])
                        vlf = wkpool.tile([128, npw], F32, tag="vlf")
                        nc.vector.tensor_copy(out=vlf[:], in_=vl[:])
                        chp = coefh_t[:].ap[0][0]
                        nc.vector.tensor_tensor(
                            out=vhf[:].rearrange("p (k c) -> p k c", c=3),
                            in0=vhf[:].rearrange("p (k c) -> p k c", c=3),
                            in1=apz(coefh_t[:], [[chp, 128], [0, KP], [1, 3]]),
                            op=TT.mult)
                        clp = coefl_t[:].ap[0][0]
                        nc.vector.tensor_tensor(
                            out=vlf[:].rearrange("p (k c) -> p k c", c=3),
                            in0=vlf[:].rearrange("p (k c) -> p k c", c=3),
                            in1=apz(coefl_t[:], [[clp, 128], [0, KP], [1, 3]]),
                            op=TT.mult)
                        hb = wkpool.tile([128, npw], I32, tag="hb")
                        nc.vector.tensor_copy(out=hb[:], in_=vhf[:])
                        lb = wkpool.tile([128, npw], I32, tag="lb")
                        nc.vector.tensor_copy(out=lb[:], in_=vlf[:])
                        nc.vector.tensor_tensor(
                            out=hb[:], in0=hb[:], in1=lb[:], op=TT.add)
                        nc.vector.tensor_scalar(
                            out=hb[:], in0=hb[:], scalar1=8191, scalar2=None,
                            op0=TT.bitwise_and)
                        hb1 = wkpool.tile([128, npw], I32, tag="hb1")
                        cap = cadd_t[:].ap[0][0]
                        nc.vector.tensor_tensor(
                            out=hb1[:].rearrange("p (k c) -> p k c", c=3),
                            in0=hb[:].rearrange("p (k c) -> p k c", c=3),
                            in1=apz(cadd_t[:], [[cap, 128], [0, KP], [1, 3]]),
                            op=TT.add)
                        nc.vector.tensor_scalar(
                            out=hb1[:], in0=hb1[:], scalar1=8191, scalar2=None,
                            op0=TT.bitwise_and)
                        if dbg_l == l and t == 0:
                            nc.sync.dma_start(out=d_v13[:], in_=v13[:])
                            nc.sync.dma_start(out=d_vh[:], in_=vh[:])
                            nc.sync.dma_start(out=d_hb[:], in_=hb[:])
                        ab = wkpool.tile([128, 4 * KP], I32, tag="ab")
                        va = [hb[:, 0::3], hb1[:, 0::3]]
                        vb = [hb[:, 1::3], hb1[:, 1::3]]
                        vc = [hb[:, 2::3], hb1[:, 2::3]]
                        for dx in range(2):
                            for dy in range(2):
                                j = dx * 2 + dy
                                nc.vector.tensor_tensor(
                                    out=ab[:, j * KP:(j + 1) * KP],
                                    in0=va[dx], in1=vb[dy],
                                    op=TT.bitwise_xor)
                        i8 = wkpool.tile([128, 8 * KP], I32, tag="i8")
                        for hw_ in range(4):
                            for dz in range(2):
                                j = hw_ * 2 + dz
                                nc.vector.tensor_tensor(
                                    out=i8[:, j * KP:(j + 1) * KP],
                                    in0=ab[:, hw_ * KP:(hw_ + 1) * KP],
                                    in1=vc[dz],
                                    op=TT.bitwise_xor)
                        nc.vector.tensor_copy(out=idxw[:], in_=i8[:])

                    # =========== replicated side: weights ===========
                    npr = GP * 3
                    posr = wkpool.tile([128, npr], F32, tag="posr")
                    nc.vector.tensor_scalar(
                        out=posr[:], in0=pr[:], scalar1=float(res - 1),
                        scalar2=None, op0=TT.mult)
                    cir = wkpool.tile([128, npr], I32, tag="cir")
                    nc.vector.tensor_copy(out=cir[:], in_=posr[:])
                    cfr = wkpool.tile([128, npr], F32, tag="cfr")
                    nc.vector.tensor_copy(out=cfr[:], in_=cir[:])
                    gr_ = wkpool.tile([128, npr], F32, tag="gr_")
                    nc.vector.tensor_tensor(
                        out=gr_[:], in0=cfr[:], in1=posr[:], op=TT.is_gt)
                    nc.vector.tensor_tensor(
                        out=cfr[:], in0=cfr[:], in1=gr_[:], op=TT.subtract)
                    nc.vector.tensor_scalar(
                        out=cfr[:], in0=cfr[:], scalar1=float(res - 2),
                        scalar2=None, op0=TT.min)
                    fr = wkpool.tile([128, npr], F32, tag="fr")
                    nc.vector.tensor_tensor(
                        out=fr[:], in0=posr[:], in1=cfr[:], op=TT.subtract)
                    wp = wkpool.tile([128, 2 * npr], F32, tag="wp")
                    nc.vector.tensor_scalar(
                        out=wp[:, :npr], in0=fr[:], scalar1=-1.0, scalar2=1.0,
                        op0=TT.mult, op1=TT.add)
                    nc.vector.tensor_copy(out=wp[:, npr:], in_=fr[:])
                    wxy = wkpool.tile([128, 4 * GP], F32, tag="wxy")
                    wpp = wp[:].ap[0][0]
                    wyp = wxy[:].ap[0][0]
                    nc.vector.tensor_tensor(
                        out=apz(wxy[:], [[wyp, 128], [2 * GP, 2], [GP, 2],
                                         [1, GP]]),
                        in0=apz(wp[:], [[wpp, 128], [npr, 2], [0, 2],
                                        [3, GP]], off=0),
                        in1=apz(wp[:], [[wpp, 128], [0, 2], [npr, 2],
                                        [3, GP]], off=1),
                        op=TT.mult)
                    w8 = wkpool.tile([128, 8 * GP], F32, tag="w8")
                    w8p = w8[:].ap[0][0]
                    nc.vector.tensor_tensor(
                        out=apz(w8[:], [[w8p, 128], [2 * GP, 4], [GP, 2],
                                        [1, GP]]),
                        in0=apz(wxy[:], [[wyp, 128], [GP, 4], [0, 2],
                                         [1, GP]]),
                        in1=apz(wp[:], [[wpp, 128], [0, 4], [npr, 2],
                                        [3, GP]], off=2),
                        op=TT.mult)
                    w8b = wkpool.tile([128, 8 * GP * 4], BF16, tag="w8b")
                    w8bp = w8b[:].ap[0][0]
                    nc.vector.tensor_copy(
                        out=apz(w8b[:], [[w8bp, 128], [4, 8 * GP], [1, 4]]),
                        in_=apz(w8[:], [[w8p, 128], [1, 8 * GP], [0, 4]]))

                    # =========== gather + blend ===========
                    gt = gpool.tile([128, NI * FEAT_DIM], BF16, tag="gt")
                    nc.gpsimd.ap_gather(
                        out_ap=gt[:], in_ap=tab_t[:], idxs_ap=idxw[:],
                        channels=128, num_elems=TABLE_SIZE, d=FEAT_DIM,
                        num_idxs=NI)
                    if dbg_l == l and t == 0:
                        nc.sync.dma_start(out=d_pw[:], in_=pw[:])
                        nc.sync.dma_start(out=d_pr[:], in_=pr[:])
                        nc.sync.dma_start(out=d_idx[:], in_=idxw[:])
                        nc.sync.dma_start(out=d_gt[:], in_=gt[:])
                        nc.sync.dma_start(out=d_w8[:], in_=w8[:])
                    nc.vector.tensor_tensor(
                        out=gt[:], in0=gt[:], in1=w8b[:], op=TT.mult)
                    half = NI * FEAT_DIM // 2
                    nc.vector.tensor_tensor(
                        out=gt[:, :half], in0=gt[:, :half], in1=gt[:, half:],
                        op=TT.add)
                    nc.vector.tensor_tensor(
                        out=gt[:, :half // 2], in0=gt[:, :half // 2],
                        in1=gt[:, half // 2:half], op=TT.add)
                    acc = gpool.tile([128, GP * FEAT_DIM], F32, tag="acc")
                    nc.vector.tensor_tensor(
                        out=acc[:], in0=gt[:, :half // 4],
                        in1=gt[:, half // 4:half // 2], op=TT.add)
                    accp = acc[:].ap[0][0]
                    nc.sync.dma_start(
                        out=feats_d[l, p0:p0 + TILE_PTS, :].rearrange(
                            "(g k) f -> g (k f)", g=8),
                        in_=apz(acc[:], [[accp * 16, 8], [1, GP * 4]]))

            # ---------------- phase B: MLP ----------------
            ident = cpool.tile([128, 128], F32)
            make_identity(nc, ident[:])
            w0_t = cpool.tile([64, 64], F32)
            nc.sync.dma_start(out=w0_t[:], in_=w0[:])
            w1_t = cpool.tile([64, 64], F32)
            nc.sync.dma_start(out=w1_t[:], in_=w1[:])
            w2_t = cpool.tile([64, 1], F32)
            nc.sync.dma_start(out=w2_t[:], in_=w2[:])
            b0_t = cpool.tile([64, 1], F32)
            nc.sync.dma_start(out=b0_t[:], in_=b0[:])
            b1_t = cpool.tile([64, 1], F32)
            nc.sync.dma_start(out=b1_t[:], in_=b1[:])
            b2_t = cpool.tile([1, 1], F32)
            nc.sync.dma_start(out=b2_t[:], in_=b2[:])

            zw = SUP // 128 * 4          # per-LOD zone width in fb (256 f32)
            for st in range(n_sup):
                q0 = st * SUP
                fb = mpool.tile([128, NUM_LODS * zw], F32, tag="fb")
                for l in range(NUM_LODS):
                    fa = feats_d[l, q0:q0 + SUP, :]
                    fbp = fb[:].ap[0][0]
                    nc.sync.dma_start(
                        out=apz(fb[:], [[fbp, 128], [64, SUP // 128], [1, 4]],
                                off=l * 4),
                        in_=apz(fa, [[4, 128], [512, SUP // 128], [1, 4]]))
                ob = mpool.tile([1, SUP], F32, tag="ob")
                for a in range(SUP // 128):
                    tp = pspool.tile([64, 128], F32, tag="tp")
                    src = fb[:, a * 64:(a + 1) * 64]
                    nc.tensor.transpose(out=tp[:], in_=src, identity=ident[:])
                    at = mpool.tile([64, 128], F32, tag="at")
                    nc.scalar.activation(
                        out=at[:], in_=tp[:],
                        func=mybir.ActivationFunctionType.Copy)
                    h1p = pspool.tile([64, 128], F32, tag="h1p")
                    nc.tensor.matmul(out=h1p[:], lhsT=w0_t[:], rhs=at[:],
                                     start=True, stop=True)
                    h1 = mpool.tile([64, 128], F32, tag="h1")
                    nc.scalar.activation(
                        out=h1[:], in_=h1p[:],
                        func=mybir.ActivationFunctionType.Relu,
                        bias=b0_t[:], scale=1.0)
                    h2p = pspool.tile([64, 128], F32, tag="h2p")
                    nc.tensor.matmul(out=h2p[:], lhsT=w1_t[:], rhs=h1[:],
                                     start=True, stop=True)
                    h2 = mpool.tile([64, 128], F32, tag="h2")
                    nc.scalar.activation(
                        out=h2[:], in_=h2p[:],
                        func=mybir.ActivationFunctionType.Relu,
                        bias=b1_t[:], scale=1.0)
                    zp = pspool.tile([1, 128], F32, tag="zp")
                    nc.tensor.matmul(out=zp[:], lhsT=w2_t[:], rhs=h2[:],
                                     start=True, stop=True)
                    nc.scalar.activation(
                        out=ob[:, a * 128:(a + 1) * 128], in_=zp[:],
                        func=mybir.ActivationFunctionType.Sigmoid,
                        bias=b2_t[:], scale=1.0)
                nc.sync.dma_start(
                    out=out[q0:q0 + SUP, :].rearrange("n f -> f n"),
                    in_=ob[:])

    nc.compile()
    return nc


_NC_CACHE = {}


def _input_maps(pts, tables, w0, b0, w1, b1, w2, b2):
    pts = np.ascontiguousarray(np.asarray(pts, dtype=np.float32))
    tabs_bf = np.asarray(tables, dtype=np.float32).reshape(
        NUM_LODS, 1, TABLE_SIZE * FEAT_DIM).astype(mybir.dt.np(BF16))
    base = {
        "tabs": tabs_bf,
        "w0": np.ascontiguousarray(np.asarray(w0, np.float32).reshape(64, 64)),
        "b0": np.ascontiguousarray(np.asarray(b0, np.float32).reshape(64, 1)),
        "w1": np.ascontiguousarray(np.asarray(w1, np.float32).reshape(64, 64)),
        "b1": np.ascontiguousarray(np.asarray(b1, np.float32).reshape(64, 1)),
        "w2": np.ascontiguousarray(np.asarray(w2, np.float32).reshape(64, 1)),
        "b2": np.ascontiguousarray(np.asarray(b2, np.float32).reshape(1, 1)),
        "coefh": np.array([[1.0, P1L & 127, P2L & 127]], np.float32),
        "coefl": np.array([[1.0, P1L, P2L]], np.float32),
        "cadd": np.array([[1, P1L, P2L]], np.int32),
        "doffs": np.array([[(dx * LODS[0] + dy) * LODS[0] + dz
                            for dx in (0, 1) for dy in (0, 1)
                            for dz in (0, 1)]], np.float32),
    }
    in_maps = []
    for c in range(N_CORES):
        m = dict(base)
        p = pts[c * N_CORE:(c + 1) * N_CORE]
        m["pts"] = p
        m["ptsw"] = np.ascontiguousarray(
            p.reshape(-1, 8, KP, 16, 3).transpose(0, 1, 3, 2, 4)
            .reshape(N_CORE, 3))
        in_maps.append(m)
    return in_maps


def kernel(pts, tables, w0, b0, w1, b1, w2, b2):
    _axon_boot()
    from concourse.bass_utils import run_bass_kernel_spmd

    if "full" not in _NC_CACHE:
        _NC_CACHE["full"] = build()
    nc = _NC_CACHE["full"]

    in_maps = _input_maps(pts, tables, w0, b0, w1, b1, w2, b2)
    trace = os.environ.get("KERNEL_TRACE", "0") == "1"
    res = run_bass_kernel_spmd(nc, in_maps, core_ids=list(range(N_CORES)),
                               trace=trace)
    if trace and res.exec_time_ns:
        print(f"HW exec time: {res.exec_time_ns} ns")
    return np.concatenate([r["out"] for r in res.results], axis=0)

